# revision 26
# baseline (speedup 1.0000x reference)
"""MoBA (Mixture of Blocks Attention) Trainium2 Bass kernel.

Problem: S=4096 tokens, H=16 query heads, HKV=4 kv heads (GQA), D=128,
chunk C=512 (nC=8), top-k=3 selected past chunks per (token, head) +
causal attention over the current chunk.

Sharding: 16 heads over 8 cores -> 2 heads/core; both heads of a core
share one kv head (head h uses kv head h//4, and heads {2c, 2c+1} give
kv head c//2). Fully independent per core, no collectives.

Per-core kernel (q [4096,256], k [4096,128], v [4096,128] -> o [4096,256]):
  - transpose q, k to [D, S] layout via PE transposes
  - gating: gates = q . mean-pooled-K per chunk (fp32), iterative top-3
    threshold on DVE, selection expressed as additive bias rows (0/-30000)
  - attention with scores computed TRANSPOSED (sT[kv, q]) per
    (q-chunk, kv-chunk) pair so PV needs no transposes of p:
      sT = k_blk^T q (f32r matmuls), selection bias added into PSUM via
      rank-1 K=1 matmuls, exp on ACT (scale fused) -> pT bf16,
      causal triangle masked post-exp via gpsimd affine_select,
      O[D, q] += v_blk^T pT (bf16 matmuls accumulated in PSUM),
      l accumulated as l_acc[kv, q] += pT on DVE then folded with a
      ones-matmul; final transpose of O back to [q, D] + 1/l scaling.
"""

import functools
import sys

import numpy as np

_REPO = "/opt/trn_rl_repo"
if _REPO not in sys.path:
    sys.path.insert(0, _REPO)

S = 4096
D = 128
C = 512
NCH = 8          # number of kv chunks
NT = S // 128    # 32 q tiles of 128 rows
HPC = 2          # heads per core
N_CORES = 8
SCALE = 1.0 / float(np.sqrt(D))
NEGB = 30000.0   # additive bias magnitude for masked-out entries


def _build_program():
    import concourse.bass as bass
    import concourse.tile as tile
    from concourse import bacc, mybir

    f32 = mybir.dt.float32
    f32r = mybir.dt.float32r
    bf16 = mybir.dt.bfloat16
    AF = mybir.ActivationFunctionType
    ALU = mybir.AluOpType
    AX = mybir.AxisListType

    nc = bacc.Bacc("TRN2", target_bir_lowering=False, debug=False,
                   num_devices=N_CORES)
    q_d = nc.declare_dram_parameter("q", [S, HPC * D], f32, isOutput=False)
    k_d = nc.declare_dram_parameter("k", [S, D], f32, isOutput=False)
    v_d = nc.declare_dram_parameter("v", [S, D], f32, isOutput=False)
    o_d = nc.declare_dram_parameter("o", [S, HPC * D], f32, isOutput=True)

    with tile.TileContext(nc) as tc:
        with tc.tile_pool(name="const", bufs=1) as cp, \
             tc.tile_pool(name="persist", bufs=1) as pp:
            # ---- constants ----
            ident = cp.tile([128, 128], f32)
            nc.gpsimd.memset(ident[:], 1.0)
            nc.gpsimd.affine_select(out=ident[:], in_=ident[:],
                                    compare_op=ALU.is_ge, fill=0.0, base=0,
                                    pattern=[[-1, 128]], channel_multiplier=1)
            nc.gpsimd.affine_select(out=ident[:], in_=ident[:],
                                    compare_op=ALU.is_ge, fill=0.0, base=0,
                                    pattern=[[1, 128]], channel_multiplier=-1)
            ones_col = cp.tile([128, 1], bf16)
            nc.gpsimd.memset(ones_col[:], 1.0)
            # row selectors, two bands so the two bias matmuls per sT tile can
            # use distinct PE row groups (tile_position (0,0) and (32,0)):
            # rows 0-7: 1 iff p == col//128; rows 32-39: 1 iff p-32 == col//128
            selrow = cp.tile([40, NCH * 128], bf16)
            nc.gpsimd.memset(selrow[:], 0.0)
            for base_p in (0, 32):
                band = selrow[base_p:base_p + 8, :]
                nc.gpsimd.memset(band, 1.0)
                nc.gpsimd.affine_select(out=band, in_=band,
                                        compare_op=ALU.is_ge, fill=0.0, base=0,
                                        pattern=[[-1, NCH], [0, 128]],
                                        channel_multiplier=1)
                nc.gpsimd.affine_select(out=band, in_=band,
                                        compare_op=ALU.is_ge, fill=0.0, base=0,
                                        pattern=[[1, NCH], [0, 128]],
                                        channel_multiplier=-1)

            # ---- persistent tensors ----
            qT = pp.tile([128, HPC * S], f32)      # [D, hh*S + s] (gating)
            qT_bf = pp.tile([128, HPC * S], bf16)  # [D, hh*S + s] (attention)
            kT = pp.tile([128, S], f32)            # [D, s] (chunk means)
            kT_bf = pp.tile([128, S], bf16)        # [D, s] (attention)
            v_bf = pp.tile([128, S], bf16)         # [s%128, 128*t + d]
            kmT = pp.tile([128, NCH], f32)         # [D, chunk]
            biasSelT = pp.tile([40, HPC * S], bf16)  # [j, hh*S + q], rows 0-7
            #                                          duplicated at rows 32-39

            # ---- load + transpose prep, gating ----
            with tc.tile_pool(name="stage", bufs=1) as sp, \
                 tc.tile_pool(name="gat", bufs=1) as gp, \
                 tc.tile_pool(name="ps_prep", bufs=2, space="PSUM") as ppp:
                q_sb = sp.tile([128, NT * HPC * D], f32)
                nc.sync.dma_start(
                    q_sb.rearrange("p (t c) -> p t c", c=HPC * D),
                    q_d.rearrange("(t p) c -> p t c", p=128))
                k_sb = sp.tile([128, NT * D], f32)
                nc.sync.dma_start(
                    k_sb.rearrange("p (t c) -> p t c", c=D),
                    k_d.rearrange("(t p) c -> p t c", p=128))
                v_sb = sp.tile([128, NT * D], f32)
                nc.sync.dma_start(
                    v_sb.rearrange("p (t c) -> p t c", c=D),
                    v_d.rearrange("(t p) c -> p t c", p=128))
                nc.vector.tensor_copy(v_bf[:], v_sb[:])

                for hh in range(HPC):
                    for g in range(NT // 4):
                        tp = ppp.tile([128, 512], f32, tag="tp")
                        for u in range(4):
                            t = 4 * g + u
                            nc.tensor.transpose(
                                tp[:, 128 * u:128 * u + 128],
                                q_sb[:, 256 * t + 128 * hh:256 * t + 128 * hh + 128],
                                ident[:])
                        nc.scalar.copy(
                            qT[:, S * hh + 512 * g:S * hh + 512 * g + 512], tp[:])
                        nc.vector.tensor_copy(
                            qT_bf[:, S * hh + 512 * g:S * hh + 512 * g + 512], tp[:])
                for g in range(NT // 4):
                    tp = ppp.tile([128, 512], f32, tag="tp")
                    for u in range(4):
                        t = 4 * g + u
                        nc.tensor.transpose(
                            tp[:, 128 * u:128 * u + 128],
                            k_sb[:, 128 * t:128 * t + 128], ident[:])
                    nc.scalar.copy(kT[:, 512 * g:512 * g + 512], tp[:])
                    nc.vector.tensor_copy(kT_bf[:, 512 * g:512 * g + 512], tp[:])

                # chunk means of K, with SCALE/C folded in
                for j in range(NCH):
                    nc.vector.tensor_reduce(
                        out=kmT[:, j:j + 1], in_=kT[:, C * j:C * j + C],
                        axis=AX.X, op=ALU.add)
                nc.vector.tensor_scalar_mul(kmT[:], kmT[:], SCALE / C)

                # ---- gating / top-3 selection ----
                for hh in range(HPC):
                    g_ps = ppp.tile([128, NT * NCH], f32, tag="g")
                    nc.vector.memset(g_ps[:], -1e30)
                    for m in range(4, NT):
                        i = m // 4
                        nc.tensor.matmul(
                            g_ps[:, NCH * m:NCH * m + i],
                            qT[:, S * hh + 128 * m:S * hh + 128 * m + 128],
                            kmT[:, 0:i], start=True, stop=True)
                    g_sb = gp.tile([128, NT * NCH], f32)
                    nc.vector.tensor_copy(g_sb[:], g_ps[:])

                    g_v = g_sb.rearrange("p (m c) -> p m c", c=NCH)
                    work = gp.tile([128, NT * NCH], f32)
                    tmp = gp.tile([128, NT * NCH], f32)
                    mx = gp.tile([128, NT], f32)
                    nc.vector.tensor_copy(work[:], g_sb[:])
                    work_v = work.rearrange("p (m c) -> p m c", c=NCH)
                    tmp_v = tmp.rearrange("p (m c) -> p m c", c=NCH)
                    for _ in range(2):  # knock out top-2
                        nc.vector.tensor_reduce(out=mx[:], in_=work_v,
                                                axis=AX.X, op=ALU.max)
                        mx_b = mx.rearrange("p (m one) -> p m one", one=1) \
                                 .broadcast_to((128, NT, NCH))
                        nc.vector.tensor_tensor(out=tmp_v, in0=work_v,
                                                in1=mx_b, op=ALU.is_equal)
                        nc.vector.tensor_scalar_mul(tmp[:], tmp[:], 2e30)
                        nc.vector.tensor_tensor(out=work[:], in0=work[:],
                                                in1=tmp[:], op=ALU.subtract)
                    nc.vector.tensor_reduce(out=mx[:], in_=work_v,
                                            axis=AX.X, op=ALU.max)
                    mx_b = mx.rearrange("p (m one) -> p m one", one=1) \
                             .broadcast_to((128, NT, NCH))
                    selA = gp.tile([128, NT * NCH], f32)
                    selA_v = selA.rearrange("p (m c) -> p m c", c=NCH)
                    nc.vector.tensor_tensor(out=selA_v, in0=g_v, in1=mx_b,
                                            op=ALU.is_ge)
                    selB = gp.tile([128, NT * NCH], f32)
                    nc.vector.tensor_scalar(out=selB[:], in0=g_sb[:],
                                            scalar1=-0.5e30, scalar2=None,
                                            op0=ALU.is_gt)
                    nc.vector.tensor_tensor(out=selA[:], in0=selA[:],
                                            in1=selB[:], op=ALU.mult)
                    biasSel = gp.tile([128, NT * NCH], f32)
                    nc.vector.tensor_scalar(out=biasSel[:], in0=selA[:],
                                            scalar1=NEGB, scalar2=NEGB,
                                            op0=ALU.mult, op1=ALU.subtract)
                    for g2 in range(NT // 8):
                        bsT = ppp.tile([8, 1024], f32, tag="bsT")
                        for u in range(8):
                            m = 8 * g2 + u
                            nc.tensor.transpose(
                                bsT[:, 128 * u:128 * u + 128],
                                biasSel[:, NCH * m:NCH * m + NCH], ident[:])
                        nc.scalar.copy(
                            biasSelT[0:8, S * hh + 1024 * g2:S * hh + 1024 * g2 + 1024],
                            bsT[:])
                        nc.scalar.copy(
                            biasSelT[32:40, S * hh + 1024 * g2:S * hh + 1024 * g2 + 1024],
                            bsT[:])

            # ---- attention ----
            with tc.tile_pool(name="ps_sT", bufs=2, space="PSUM") as ps_sT, \
                 tc.tile_pool(name="ps_O", bufs=1, space="PSUM") as ps_O, \
                 tc.tile_pool(name="ps_misc", bufs=3, space="PSUM") as ps_misc, \
                 tc.tile_pool(name="att", bufs=1) as ap_att, \
                 tc.tile_pool(name="pTp", bufs=4) as ap_pT, \
                 tc.tile_pool(name="lap", bufs=2) as ap_l, \
                 tc.tile_pool(name="osb", bufs=2) as ap_o:
                for hh in range(HPC):
                    for i in range(NCH):
                        O_ps = ps_O.tile([128, 512], f32, tag="O")
                        l_acc = ap_l.tile([128, 4 * C], bf16, tag="lacc")
                        for j in range(i + 1):
                            for h2 in range(2):
                                sT = ps_sT.tile([128, 1024], f32, tag="sT")
                                pT = ap_pT.tile([128, 1024], bf16, tag="pT")
                                if j < i:
                                    for kbl in range(2):
                                        kb = 2 * h2 + kbl
                                        nc.tensor.matmul(
                                            sT[:, 512 * kbl:512 * kbl + 512],
                                            kT_bf[:, C * j + 128 * kb:C * j + 128 * kb + 128],
                                            qT_bf[:, S * hh + C * i:S * hh + C * i + 512],
                                            start=True, stop=False)
                                    # rank-1 selection-bias adds; distinct row
                                    # groups so the two K=8 matmuls overlap
                                    for kbl in range(2):
                                        bp = 32 * kbl
                                        nc.tensor.matmul(
                                            sT[:, 512 * kbl:512 * kbl + 512],
                                            selrow[bp:bp + 8, 128 * j:128 * j + 128],
                                            biasSelT[bp:bp + 8, S * hh + C * i:S * hh + C * i + 512],
                                            start=False, stop=True,
                                            tile_position=(bp, 0))
                                    nc.scalar.activation(pT[:], sT[:], AF.Exp,
                                                         scale=SCALE)
                                    if j == 0:
                                        # first contribution: copy (4x DVE mode)
                                        nc.vector.tensor_copy(
                                            l_acc[:, 1024 * h2:1024 * h2 + 1024],
                                            pT[:])
                                    else:
                                        nc.vector.tensor_tensor(
                                            out=l_acc[:, 1024 * h2:1024 * h2 + 1024],
                                            in0=l_acc[:, 1024 * h2:1024 * h2 + 1024],
                                            in1=pT[:], op=ALU.add)
                                    for kbl in range(2):
                                        kb = 2 * h2 + kbl
                                        nc.tensor.matmul(
                                            O_ps[:],
                                            v_bf[:, 128 * (4 * j + kb):128 * (4 * j + kb) + 128],
                                            pT[:, 512 * kbl:512 * kbl + 512],
                                            start=(j == 0 and kb == 0), stop=False)
                                else:
                                    for kbl in range(2):
                                        kb = 2 * h2 + kbl
                                        lo = 512 * kbl + 128 * kb
                                        hi = 512 * kbl + 512
                                        nc.tensor.matmul(
                                            sT[:, lo:hi],
                                            kT_bf[:, C * j + 128 * kb:C * j + 128 * kb + 128],
                                            qT_bf[:, S * hh + C * i + 128 * kb:S * hh + C * i + 512],
                                            start=True, stop=True)
                                        nc.scalar.activation(pT[:, lo:hi],
                                                             sT[:, lo:hi],
                                                             AF.Exp, scale=SCALE)
                                        # causal triangle on the leading 128 cols
                                        nc.gpsimd.affine_select(
                                            out=pT[:, lo:lo + 128],
                                            in_=pT[:, lo:lo + 128],
                                            compare_op=ALU.is_ge, fill=0.0,
                                            base=0, pattern=[[1, 128]],
                                            channel_multiplier=-1)
                                        if j == 0:
                                            nc.vector.tensor_copy(
                                                l_acc[:, C * kb + 128 * kb:C * kb + 512],
                                                pT[:, lo:hi])
                                        else:
                                            nc.vector.tensor_tensor(
                                                out=l_acc[:, C * kb + 128 * kb:C * kb + 512],
                                                in0=l_acc[:, C * kb + 128 * kb:C * kb + 512],
                                                in1=pT[:, lo:hi], op=ALU.add)
                                        nc.tensor.matmul(
                                            O_ps[:, 128 * kb:512],
                                            v_bf[:, 128 * (4 * j + kb):128 * (4 * j + kb) + 128],
                                            pT[:, lo:hi],
                                            start=(i == 0 and kb == 0),
                                            stop=(kb == 3))

                        # ---- l, normalize, output ----
                        l_ps = ps_misc.tile([1, 512], f32, tag="misc")
                        for kb in range(4):
                            # for i==0 only the causally-valid part of l_acc
                            # was ever written; read just that slice
                            off = 128 * kb if i == 0 else 0
                            nc.tensor.matmul(l_ps[:, off:512], ones_col[:],
                                             l_acc[:, C * kb + off:C * kb + 512],
                                             start=(kb == 0), stop=(kb == 3))
                        l_sb = ap_att.tile([1, 512], f32, tag="lsb", bufs=2)
                        nc.scalar.copy(l_sb[:], l_ps[:])
                        linvT = ps_misc.tile([128, 4], f32, tag="misc")
                        for qs in range(4):
                            nc.tensor.transpose(linvT[:, qs:qs + 1],
                                                l_sb[0:1, 128 * qs:128 * qs + 128],
                                                ident[0:1, 0:1])
                        linv = ap_att.tile([128, 4], f32, tag="linv", bufs=2)
                        nc.vector.reciprocal(linv[:], linvT[:])
                        O_sb = ap_o.tile([128, 512], f32, tag="Osb")
                        nc.scalar.copy(O_sb[:], O_ps[:])
                        outT = ps_misc.tile([128, 512], f32, tag="misc")
                        for qs in range(4):
                            nc.tensor.transpose(outT[:, 128 * qs:128 * qs + 128],
                                                O_sb[:, 128 * qs:128 * qs + 128],
                                                ident[:])
                        onrm = ap_o.tile([128, 512], f32, tag="onrm")
                        for qs in range(4):
                            nc.vector.tensor_scalar(
                                out=onrm[:, 128 * qs:128 * qs + 128],
                                in0=outT[:, 128 * qs:128 * qs + 128],
                                scalar1=linv[:, qs:qs + 1], scalar2=None,
                                op0=ALU.mult)
                        nc.sync.dma_start(
                            o_d[C * i:C * i + C, 128 * hh:128 * hh + 128]
                               .rearrange("(qs p) d -> p qs d", p=128),
                            onrm.rearrange("p (qs d) -> p qs d", d=128))

    nc.compile()
    return nc


@functools.lru_cache(maxsize=1)
def _compiled():
    return _build_program()


def shard_inputs(query, key, value):
    q = np.ascontiguousarray(np.asarray(query, dtype=np.float32))
    k = np.ascontiguousarray(np.asarray(key, dtype=np.float32))
    v = np.ascontiguousarray(np.asarray(value, dtype=np.float32))
    in_maps = []
    for c in range(N_CORES):
        h0 = HPC * c
        kvh = h0 // 4
        in_maps.append({
            "q": np.ascontiguousarray(q[:, D * h0:D * h0 + HPC * D]),
            "k": np.ascontiguousarray(k[:, D * kvh:D * kvh + D]),
            "v": np.ascontiguousarray(v[:, D * kvh:D * kvh + D]),
        })
    return in_maps


def kernel(query, key, value):
    from concourse.bass_utils import run_bass_kernel_spmd
    nc = _compiled()
    in_maps = shard_inputs(query, key, value)
    res = run_bass_kernel_spmd(nc, in_maps, core_ids=list(range(N_CORES)))
    out = np.concatenate([res.results[c]["o"] for c in range(N_CORES)], axis=1)
    return out.astype(np.float32)
